# revision 21
# baseline (speedup 1.0000x reference)
"""Conformer encoder layer on 8 Trainium2 NeuronCores.

Sharding: pure data-parallel over batch N=16 -> 2 batches/core, no collectives.
Layout: activations transposed (features on partitions, time on free dim);
weights host-pre-transposed to (in_features, out_features), resident in SBUF
(except macaron-FFN weights, streamed per rep), loaded with one DMA per tensor.
Precision: fp16 matmul operands with fp32 PSUM accumulation.

vs the v1 kernel:
- pos_emb @ pos_w precomputed on host (constant across reps), resident.
- rel-shift DRAM round-trip batched per (batch, head): 1 write + 1 diagonal
  3D-AP read instead of 8 DMAs.
- ac+bd add done on the PE via an identity matmul accumulating the shifted
  bd into the ac PSUM bank (no DVE add, exp reads PSUM directly).
- A^T for the A@V matmul via PE transposes + Pool-engine PSUM->SBUF copies
  (no DMA xbar transpose).
- depthwise conv as 31 shifted MACs on the DVE (fp16, 4x mode) instead of
  31 diagonal matmuls on the PE; no 4MB diag weight tensor.
- y store per 512-col tile, batched weight loads, fewer/larger DMAs.
"""
import sys
sys.path.insert(0, '/opt/trn_rl_repo')
import numpy as np

T, N, E, H, DFF, KC = 512, 16, 512, 8, 2048, 31
D = E // H          # 64
NB = 2              # batches per core
NCORE = 8
PAD = (KC - 1) // 2  # 15

_cached = {}


def pr_of(hh):
    return slice(hh * 64, hh * 64 + 64)


def _build(repeat=1):
    import bass_rust
    import concourse.bass as bass
    import concourse.bacc as bacc
    import concourse.mybir as mybir
    import concourse.tile as tile

    dt = mybir.dt
    Alu = mybir.AluOpType
    Act = mybir.ActivationFunctionType
    ts = bass.ts
    F32, F32R, F16 = dt.float32, dt.float32r, dt.float16

    nc = bacc.Bacc("TRN2", target_bir_lowering=False, debug=False)

    def din(name, shape, dtype=F32):
        return nc.dram_tensor(name, list(shape), dtype,
                              kind="ExternalInput").ap()

    xt_d = din("xt", (NB, E, T), F16)
    pT_d = din("pT", (E, 1024), F16)
    w_ffm1_d = din("w_ffm1", (E, DFF), F16)
    bf1_d = din("bf1", (128, 16))
    bf1m_d = din("bf1m", (128, 16))
    w_ffm2_d = din("w_ffm2", (DFF, E), F16)
    bf2_d = din("bf2", (128, 4))
    w_q_d = din("w_q", (E, E), F16)
    w_k_d = din("w_k", (E, E), F16)
    w_v_d = din("w_v", (E, E), F16)
    bq_d = din("bq", (128, 4))
    bk_d = din("bk", (128, 4))
    dvu_d = din("dvu", (128, 4))
    bv_row_d = din("bv_row", (1, E), F16)
    w_out_d = din("w_out", (E, E), F16)
    bo_d = din("bo", (128, 4))
    w_pw1_d = din("w_pw1", (E, 2 * E), F16)
    bpa_d = din("bpa", (128, 4))
    bpb_d = din("bpb", (128, 4))
    dwcol_d = din("dwcol", (128, 4 * KC))
    bdw_d = din("bdw", (128, 4))
    bdwm_d = din("bdwm", (128, 4))
    w_pw2_d = din("w_pw2", (E, E), F16)
    bp2_d = din("bp2", (128, 4))
    w_ff1_d = din("w_ff1", (E, DFF), F16)
    bg1_d = din("bg1", (128, 16))
    bg1m_d = din("bg1m", (128, 16))
    w_ff2_d = din("w_ff2", (DFF, E), F16)
    bg2_d = din("bg2", (128, 4))
    eps_d = din("eps_c", (1, 1))
    ones16_d = din("ones16", (1, 128), F16)
    onescol16_d = din("onescol16", (128, 1), F16)
    ones32_d = din("ones32", (1, 128))
    ident16_d = din("ident16", (128, 128), F16)

    yt_d = nc.dram_tensor("yt", [NB, E, T], F32, kind="ExternalOutput").ap()

    # per-(n, h) rel-shift scratch in DRAM: 4 t-tiles of [128, 640] each
    bds_d = [[nc.dram_tensor(f"bds_{n}_{h}", [128, 4 * 640], F16,
                             kind="Internal").ap()
              for h in range(H)] for n in range(NB)]

    def diag4_ap(d_ap):
        # read[p, tt, j] = tile_tt[p, 127 + j - p]
        a = d_ap.flatten().copy()
        a.ap = bass_rust.VecI64Pair([[2559, 128], [640, 4], [1, 512]])
        a.offset = 127
        return a

    def r3(ap2d):
        # (E-like, F) dram -> (128, a, F) partition view
        return ap2d.rearrange("(a p) f -> p a f", p=128)

    with tile.TileContext(nc) as tc:
        cpool_ctx = tc.tile_pool(name="consts", bufs=1)
        cpool = cpool_ctx.__enter__()
        wts_ctx = tc.tile_pool(name="wts", bufs=1)
        wts = wts_ctx.__enter__()
        apool_ctx = tc.tile_pool(name="acts", bufs=1)
        ap = apool_ctx.__enter__()
        psum_ctx = tc.tile_pool(name="psum", bufs=1, space="PSUM")
        psum = psum_ctx.__enter__()

        def pwork(name):
            return psum.tile([128, 512], F32, tag="work", bufs=4, name=name)

        def pacc(name):
            return psum.tile([128, 512], F32, tag="acc", bufs=4, name=name)

        # ---- constants ----
        def cload(name, d_ap, shape, dtype=F32):
            t_ = cpool.tile(list(shape), dtype, name=name)
            nc.sync.dma_start(t_[:], d_ap if dtype != F32R
                              else d_ap.bitcast(F32R))
            return t_

        bf1_sb = cload("bf1_sb", bf1_d, (128, 16))
        bf1m_sb = cload("bf1m_sb", bf1m_d, (128, 16))
        bf2_sb = cload("bf2_sb", bf2_d, (128, 4))
        bq_sb = cload("bq_sb", bq_d, (128, 4))
        bk_sb = cload("bk_sb", bk_d, (128, 4))
        dvu_sb = cload("dvu_sb", dvu_d, (128, 4))
        bv_row_sb = cload("bv_row_sb", bv_row_d, (1, E), F16)
        bo_sb = cload("bo_sb", bo_d, (128, 4))
        bpa_sb = cload("bpa_sb", bpa_d, (128, 4))
        bpb_sb = cload("bpb_sb", bpb_d, (128, 4))
        dwcol_sb = cload("dwcol_sb", dwcol_d, (128, 4 * KC))
        bdw_sb = cload("bdw_sb", bdw_d, (128, 4))
        bdwm_sb = cload("bdwm_sb", bdwm_d, (128, 4))
        bp2_sb = cload("bp2_sb", bp2_d, (128, 4))
        bg1_sb = cload("bg1_sb", bg1_d, (128, 16))
        bg1m_sb = cload("bg1m_sb", bg1m_d, (128, 16))
        bg2_sb = cload("bg2_sb", bg2_d, (128, 4))
        eps_sb = cload("eps_sb", eps_d, (1, 1))
        ones16_sb = cload("ones16_sb", ones16_d, (1, 128), F16)
        onescol16_sb = cload("onescol16_sb", onescol16_d, (128, 1), F16)
        ones32r_sb = cload("ones32r_sb", ones32_d, (1, 128), F32R)
        ident16_sb = cload("ident16_sb", ident16_d, (128, 128), F16)

        # ---- resident weights (one DMA per tensor) ----
        def wres(name, d_ap, a, f):
            t_ = wts.tile([128, a, f], F16, name=name)
            nc.sync.dma_start(t_[:], r3(d_ap))
            return t_

        pT_sb = wres("pT_sb", pT_d, 4, 1024)
        wq_sb = wres("wq_sb", w_q_d, 4, 512)
        wk_sb = wres("wk_sb", w_k_d, 4, 512)
        wv_sb = wres("wv_sb", w_v_d, 4, 512)
        wo_sb = wres("wo_sb", w_out_d, 4, 512)
        wpw1_sb = wres("wpw1_sb", w_pw1_d, 4, 1024)
        wpw2_sb = wres("wpw2_sb", w_pw2_d, 4, 512)

        # ---- per-repetition body ----
        def emit_rep():
            def xtile(n, stage):
                return ap.tile([128, 4, 512], F16, tag=f"x{n}", bufs=2,
                               name=f"x{stage}_{n}")

            x_cur = []
            for n in range(NB):
                x0 = xtile(n, 0)
                nc.sync.dma_start(x0[:], r3(xt_d[n]))
                x_cur.append(x0)

            # FFN weights streamed per rep through shared buffers
            def ffw_load(w1_d, w2_d, pref):
                w1 = ap.tile([128, 4, DFF], F16, tag="wf1", bufs=2,
                             name=f"{pref}w1")
                nc.sync.dma_start(w1[:], r3(w1_d))
                w2 = ap.tile([128, 16, 512], F16, tag="wf2", bufs=1,
                             name=f"{pref}w2")
                nc.sync.dma_start(w2[:], r3(w2_d))
                return w1, w2

            wffm1, wffm2 = ffw_load(w_ffm1_d, w_ffm2_d, "ffm")

            # ---- FFN (macaron + final) ----
            def ffn(tag, w1_sb, b1, b1m, w2_sb, b2, stage, only_n=None,
                    filler=None):
                for n in (range(NB) if only_n is None else [only_n]):
                    xin = x_cur[n]
                    accs = [pacc(f"{tag}acc{n}_{et}") for et in range(4)]
                    sds = []

                    def h2_emit(d):
                        for et in range(4):
                            nc.tensor.matmul(
                                accs[et][:], w2_sb[:, d, ts(et, 128)],
                                sds[d][:], start=(d == 0), stop=(d == 15))

                    for d in range(16):
                        hps = pwork(f"{tag}h1_{n}_{d}")
                        for et in range(4):
                            nc.tensor.matmul(
                                hps[:], w1_sb[:, et, ts(d, 128)],
                                xin[:, et, :],
                                start=(et == 0), stop=(et == 3))
                        sg = ap.tile([128, 512], F16, tag="ffsg", bufs=3,
                                     name=f"{tag}sg{n}{d}")
                        nc.scalar.activation(sg[:], hps[:], Act.Sigmoid,
                                             bias=b1m[:, d:d + 1])
                        sd = ap.tile([128, 512], F16, tag="ffsd", bufs=4,
                                     name=f"{tag}sd{n}{d}")
                        nc.vector.scalar_tensor_tensor(
                            sd[:], hps[:], b1[:, d:d + 1], sg[:],
                            op0=Alu.add, op1=Alu.mult)
                        sds.append(sd)
                        if d >= 1:
                            h2_emit(d - 1)
                        if filler is not None:
                            filler()
                    h2_emit(15)
                    xo = xtile(n, stage)
                    for et in range(4):
                        nc.vector.scalar_tensor_tensor(
                            xo[:, et, :], accs[et][:], b2[:, et:et + 1],
                            xin[:, et, :], op0=Alu.add, op1=Alu.add)
                    x_cur[n] = xo

            ffn("ffm", wffm1, bf1_sb, bf1m_sb, wffm2, bf2_sb, 1)
            wff1, wff2 = ffw_load(w_ff1_d, w_ff2_d, "ff2")

            # ---- attention: q/k/v projections ----
            q_sb, k_sb, v_sb, oT_sb = [], [], [], []
            for n in range(NB):
                x1 = x_cur[n]
                q_ = ap.tile([128, 4, 512], F16, tag=f"q{n}", bufs=1,
                             name=f"q_{n}")
                k_ = ap.tile([128, 4, 512], F16, tag=f"k{n}", bufs=1,
                             name=f"k_{n}")
                v_ = ap.tile([128, 4, 512], F16, tag=f"v{n}", bufs=1,
                             name=f"v_{n}")
                for i in range(4):
                    qps = pwork(f"qps{n}{i}")
                    for et in range(4):
                        nc.tensor.matmul(qps[:], wq_sb[:, et, ts(i, 128)],
                                         x1[:, et, :],
                                         start=(et == 0), stop=(et == 3))
                    nc.scalar.activation(q_[:, i, :], qps[:], Act.Identity,
                                         bias=bq_sb[:, i:i + 1])
                    kps = pwork(f"kps{n}{i}")
                    for et in range(4):
                        nc.tensor.matmul(kps[:], wk_sb[:, et, ts(i, 128)],
                                         x1[:, et, :],
                                         start=(et == 0), stop=(et == 3))
                    nc.scalar.activation(k_[:, i, :], kps[:], Act.Identity,
                                         bias=bk_sb[:, i:i + 1])
                for tt in range(4):
                    vps = pwork(f"vps{n}{tt}")
                    for et in range(4):
                        nc.tensor.matmul(vps[:], x1[:, et, ts(tt, 128)],
                                         wv_sb[:, et, :], start=(et == 0),
                                         stop=False)
                    nc.tensor.matmul(vps[:], ones16_sb[:], bv_row_sb[:],
                                     start=False, stop=True)
                    nc.scalar.activation(v_[:, tt, :], vps[:], Act.Copy)
                q_sb.append(q_)
                k_sb.append(k_)
                v_sb.append(v_)
                oT_sb.append(ap.tile([128, 4, 512], F16, tag=f"oT{n}",
                                     bufs=1, name=f"oT_{n}"))

            # ---- conv module: pre (pw1+glu+MAC k=0) / rounds / post ----
            conv_state = {}

            def conv_pre(n):
                x2 = x_cur[n]
                glus, dwas = [], []
                for cf in range(4):
                    bps = pwork(f"glb{n}{cf}")
                    for et in range(4):
                        nc.tensor.matmul(bps[:],
                                         wpw1_sb[:, et, ts(cf + 4, 128)],
                                         x2[:, et, :],
                                         start=(et == 0), stop=(et == 3))
                    sgl = ap.tile([128, 512], F16, tag="cvsg", bufs=2,
                                  name=f"cvsg{n}{cf}")
                    nc.scalar.activation(sgl[:], bps[:], Act.Sigmoid,
                                         bias=bpb_sb[:, cf:cf + 1])
                    aps = pwork(f"gla{n}{cf}")
                    for et in range(4):
                        nc.tensor.matmul(aps[:],
                                         wpw1_sb[:, et, ts(cf, 128)],
                                         x2[:, et, :],
                                         start=(et == 0), stop=(et == 3))
                    glu = ap.tile([128, 542], F16, tag="glu", bufs=4,
                                  name=f"glu{n}{cf}")
                    nc.gpsimd.memset(glu[:, 0:PAD], 0.0)
                    nc.gpsimd.memset(glu[:, 527:542], 0.0)
                    nc.vector.scalar_tensor_tensor(
                        glu[:, PAD:527], aps[:], bpa_sb[:, cf:cf + 1],
                        sgl[:], op0=Alu.add, op1=Alu.mult)
                    dwa = ap.tile([128, 512], F16, tag="dwa", bufs=4,
                                  name=f"dwa{n}{cf}")
                    wc = dwcol_sb[:, cf * KC:cf * KC + 1]
                    nc.vector.tensor_scalar_mul(dwa[:], glu[:, 0:512], wc)
                    glus.append(glu)
                    dwas.append(dwa)
                conv_state[n] = {'glus': glus, 'dwas': dwas, 'k': 1,
                                 'x2': x2}

            def conv_rounds(n, count, act_scales=False):
                st = conv_state.get(n)
                if st is None:
                    return
                while count > 0 and st['k'] < KC:
                    k_ = st['k']
                    for cf in range(4):
                        glu, dwa = st['glus'][cf], st['dwas'][cf]
                        wc = dwcol_sb[:, cf * KC + k_:cf * KC + k_ + 1]
                        tmp = ap.tile([128, 512], F16, tag="dwt", bufs=3,
                                      name=f"dwt{n}{cf}{k_}")
                        if act_scales and cf < 2:
                            nc.scalar.activation(tmp[:], glu[:, k_:k_ + 512],
                                                 Act.Copy, scale=wc)
                        else:
                            nc.vector.tensor_scalar_mul(
                                tmp[:], glu[:, k_:k_ + 512], wc)
                        nc.vector.tensor_add(dwa[:], dwa[:], tmp[:])
                    st['k'] += 1
                    count -= 1

            def conv_post(n):
                conv_rounds(n, KC)
                st = conv_state.pop(n)
                x2 = st['x2']
                ys = []
                for cf in range(4):
                    dwa = st['dwas'][cf]
                    sg2 = ap.tile([128, 512], F16, tag="cvsg", bufs=2,
                                  name=f"dwsg{n}{cf}")
                    nc.scalar.activation(sg2[:], dwa[:], Act.Sigmoid,
                                         bias=bdwm_sb[:, cf:cf + 1])
                    y_ = ap.tile([128, 512], F16, tag="ydw", bufs=4,
                                 name=f"ydw{n}{cf}")
                    nc.vector.scalar_tensor_tensor(
                        y_[:], dwa[:], bdw_sb[:, cf:cf + 1], sg2[:],
                        op0=Alu.add, op1=Alu.mult)
                    ys.append(y_)
                x3 = xtile(n, 3)
                for of in range(4):
                    cps = pacc(f"pw2{n}{of}")
                    for cf in range(4):
                        nc.tensor.matmul(cps[:],
                                         wpw2_sb[:, cf, ts(of, 128)],
                                         ys[cf][:],
                                         start=(cf == 0), stop=(cf == 3))
                    nc.vector.scalar_tensor_tensor(
                        x3[:, of, :], cps[:], bp2_sb[:, of:of + 1],
                        x2[:, of, :], op0=Alu.add, op1=Alu.add)
                x_cur[n] = x3

            # ---- attention: produce / consume pipeline ----
            bdsh_t = {}
            ops_t = {}

            def produce(u):
                n, h = u
                hp, hh = h // 2, h % 2
                pr = pr_of(hh)
                tpos = (hh * 64, 0)
                qvu = ap.tile([128, 512], F16, tag="qvu", bufs=2,
                              name=f"qvu{n}{h}")
                nc.scalar.activation(qvu[pr, :], q_sb[n][pr, hp, :],
                                     Act.Identity,
                                     bias=dvu_sb[pr, hp:hp + 1])
                bd_sb = ap.tile([128, 4, 640], F16, tag="bdsb", bufs=2,
                                name=f"bdsb{n}{h}")
                bdBt = psum.tile([128, 4, 128], F32, tag="acc", bufs=4,
                                 name=f"bdB{n}{h}")
                for tt in range(4):
                    w0 = 384 - tt * 128
                    bdA = pwork(f"bdA{n}{h}{tt}")
                    nc.tensor.matmul(
                        bdA[:], qvu[pr, ts(tt, 128)],
                        pT_sb[pr, hp, w0:w0 + 512],
                        start=True, stop=True, tile_position=tpos)
                    nc.tensor.matmul(
                        bdBt[:, tt, :], qvu[pr, ts(tt, 128)],
                        pT_sb[pr, hp, w0 + 512:w0 + 640],
                        start=True, stop=True, tile_position=tpos)
                    if n == 0:
                        nc.vector.tensor_copy(bd_sb[:, tt, 0:512], bdA[:])
                    else:
                        nc.scalar.activation(bd_sb[:, tt, 0:512], bdA[:],
                                             Act.Copy)
                nc.scalar.activation(bd_sb[:, :, 512:640], bdBt[:],
                                     Act.Copy)
                nc.sync.dma_start(bds_d[n][h], bd_sb[:])
                bdsh = ap.tile([128, 4, 512], F16, tag="bdsh", bufs=2,
                               name=f"bdsh{n}{h}")
                nc.sync.dma_start(bdsh[:], diag4_ap(bds_d[n][h]))
                bdsh_t[u] = bdsh

            def consume(u):
                n, h = u
                hp, hh = h // 2, h % 2
                pr = pr_of(hh)
                tpos = (hh * 64, 0)
                bdsh = bdsh_t.pop(u)
                if hh == 0:
                    ops_t[(n, hp)] = pacc(f"ops{n}{hp}")
                at_t = ap.tile([128, 4, 512], F16, tag="at", bufs=2,
                               name=f"at{n}{h}")
                a_ts = []
                for tt in range(4):
                    acps = pwork(f"ac{n}{h}{tt}")
                    nc.tensor.matmul(
                        acps[:], q_sb[n][pr, hp, ts(tt, 128)],
                        k_sb[n][pr, hp, :],
                        start=True, stop=False, tile_position=tpos)
                    nc.tensor.matmul(
                        acps[:], ident16_sb[:], bdsh[:, tt, :],
                        start=False, stop=True)
                    e_t = ap.tile([128, 512], F16, tag="esb", bufs=4,
                                  name=f"e{n}{h}{tt}")
                    zz = ap.tile([128, 1], F32, tag="z", bufs=8,
                                 name=f"z{n}{h}{tt}")
                    nc.scalar.activation(e_t[:], acps[:], Act.Exp,
                                         accum_out=zz[:])
                    rz = ap.tile([128, 1], F32, tag="rz", bufs=8,
                                 name=f"rz{n}{h}{tt}")
                    nc.vector.reciprocal(rz[:], zz[:])
                    a_t = ap.tile([128, 512], F16, tag="asb", bufs=4,
                                  name=f"a{n}{h}{tt}")
                    nc.vector.tensor_scalar_mul(a_t[:], e_t[:], rz[:, 0:1])
                    a_ts.append(a_t)
                for tt in range(4):
                    tp = psum.tile([128, 4, 128], F16, tag="work", bufs=4,
                                   name=f"tp{n}{h}{tt}")
                    for b in range(4):
                        nc.tensor.transpose(tp[:, b, :],
                                            a_ts[tt][:, ts(b, 128)],
                                            ident16_sb[:])
                    if n == 0:
                        nc.vector.tensor_copy(at_t[:, :, ts(tt, 128)], tp[:])
                    else:
                        nc.scalar.activation(at_t[:, :, ts(tt, 128)], tp[:],
                                             Act.Copy)
                ops_ = ops_t[(n, hp)]
                for st in range(4):
                    nc.tensor.matmul(
                        ops_[pr, :], v_sb[n][:, st, h * 64:h * 64 + 64],
                        at_t[:, st, :], start=(st == 0), stop=(st == 3),
                        tile_position=(0, hh * 64))
                if hh == 1:
                    nc.scalar.activation(oT_sb[n][:, hp, :],
                                         ops_t.pop((n, hp))[:], Act.Copy)
                if hp == 3 and hh == 1:
                    oproj(n)

            def oproj(n):
                x2 = xtile(n, 2)
                for of in range(4):
                    pps = pwork(f"oproj{n}{of}")
                    for hp in range(4):
                        nc.tensor.matmul(pps[:], wo_sb[:, hp, ts(of, 128)],
                                         oT_sb[n][:, hp, :],
                                         start=(hp == 0), stop=(hp == 3))
                    nc.vector.scalar_tensor_tensor(
                        x2[:, of, :], pps[:], bo_sb[:, of:of + 1],
                        x_cur[n][:, of, :], op0=Alu.add, op1=Alu.add)
                x_cur[n] = x2

            LAG = 2
            units = [(n, h) for n in range(NB) for h in range(H)]

            def post_consume(u):
                if u == (0, H - 1):
                    conv_pre(0)

            for i, u in enumerate(units):
                produce(u)
                if i >= LAG:
                    consume(units[i - LAG])
                    post_consume(units[i - LAG])
                    conv_rounds(0, 3)
            for i in range(len(units) - LAG, len(units)):
                consume(units[i])
                post_consume(units[i])
                conv_rounds(0, 3)

            # rep tail: n=0 conv finishes, ffn2(0) overlaps n=1 conv MACs
            conv_post(0)
            conv_pre(1)
            ffn("ff2", wff1, bg1_sb, bg1m_sb, wff2, bg2_sb, 4, only_n=0,
                filler=lambda: conv_rounds(1, 2, act_scales=True))

            # ---- BasicNorm + output ----
            yt_r = [r3(yt_d[n]) for n in range(NB)]

            def norm(n):
                x4 = x_cur[n]
                msps = psum.tile([1, 512], F32, tag="work", bufs=4,
                                 name=f"ms{n}")
                for et in range(4):
                    sq = ap.tile([128, 512], F16, tag="sq", bufs=1,
                                 name=f"sq{n}{et}")
                    nc.vector.tensor_mul(sq[:], x4[:, et, :], x4[:, et, :])
                    nc.tensor.matmul(msps[:], onescol16_sb[:], sq[:],
                                     start=(et == 0), stop=(et == 3))
                sc1 = ap.tile([1, 512], F32, tag="sc1", bufs=1,
                              name=f"sc1{n}")
                nc.scalar.activation(sc1[:], msps[:], Act.Sqrt,
                                     bias=eps_sb[0:1, 0:1], scale=1.0 / E)
                rsc = ap.tile([1, 512], F32, tag="rsc", bufs=1,
                              name=f"rsc{n}")
                nc.vector.reciprocal(rsc[:], sc1[:])
                rscr = ap.tile([1, 512], F32R, tag="rscr", bufs=1,
                               name=f"rscr{n}")
                nc.vector.tensor_copy(rscr[:], rsc[:])
                bcps = pacc(f"bc{n}")
                nc.tensor.matmul(bcps[:], ones32r_sb[:], rscr[:],
                                 start=True, stop=True)
                for et in range(4):
                    yo = ap.tile([128, 512], F32, tag="yo", bufs=2,
                                 name=f"yo{n}{et}")
                    nc.vector.tensor_mul(yo[:], x4[:, et, :], bcps[:])
                    nc.sync.dma_start(yt_r[n][:, et, :], yo[:])

            norm(0)
            conv_post(1)
            ffn("ff2b", wff1, bg1_sb, bg1m_sb, wff2, bg2_sb, 4, only_n=1)
            norm(1)

        for _rep in range(repeat):
            emit_rep()

        psum_ctx.__exit__(None, None, None)
        apool_ctx.__exit__(None, None, None)
        wts_ctx.__exit__(None, None, None)
        cpool_ctx.__exit__(None, None, None)

    nc.compile()
    return nc


def _prep_inputs(inputs):
    f32 = np.float32
    f16 = np.float16
    s = np.float32(D ** -0.5)
    src = np.asarray(inputs['src'], f32)
    pos_emb = np.asarray(inputs['pos_emb'], f32)
    ipw = np.asarray(inputs['in_proj_w'], f32)
    ipb = np.asarray(inputs['in_proj_b'], f32)
    bu = np.asarray(inputs['pos_bias_u'], f32).reshape(E)
    bv = np.asarray(inputs['pos_bias_v'], f32).reshape(E)

    def t_(a):
        return np.ascontiguousarray(np.asarray(a, f32).T.astype(f16))

    def btile(b):  # (F,) -> (128, F//128) with [p, i] = b[i*128+p]
        b = np.asarray(b, f32)
        return np.ascontiguousarray(b.reshape(-1, 128).T)

    # host-precomputed position projection: pT[e, m] = (pos_emb @ pos_w.T)[m, e]
    pos_p = pos_emb[0] @ np.asarray(inputs['pos_w'], f32).T  # (2T-1, E)
    pT = np.zeros((E, 1024), f16)
    pT[:, :2 * T - 1] = pos_p.T.astype(f16)

    dw = np.asarray(inputs['conv_dw_w'], f32).reshape(E, KC)
    dwr = dw.reshape(4, 128, KC).transpose(1, 0, 2)      # (128p, 4cf, 31k)
    dwcol = np.ascontiguousarray(dwr.reshape(128, 4 * KC))

    common = {
        'pT': pT,
        'w_ffm1': t_(inputs['ffm_w1']), 'bf1': btile(inputs['ffm_b1']),
        'bf1m': btile(np.asarray(inputs['ffm_b1'], f32) - 1.0),
        'w_ffm2': t_(inputs['ffm_w2']), 'bf2': btile(inputs['ffm_b2']),
        'w_q': np.ascontiguousarray((ipw[0:E] * s).T.astype(f16)),
        'w_k': t_(ipw[E:2 * E]), 'w_v': t_(ipw[2 * E:3 * E]),
        'bq': btile(ipb[0:E] * s + bu), 'bk': btile(ipb[E:2 * E]),
        'dvu': btile(bv - bu),
        'bv_row': np.ascontiguousarray(
            ipb[2 * E:3 * E].reshape(1, E).astype(f16)),
        'w_out': t_(inputs['out_w']), 'bo': btile(inputs['out_b']),
        'w_pw1': t_(inputs['conv_pw1_w']),
        'bpa': btile(np.asarray(inputs['conv_pw1_b'], f32)[0:E]),
        'bpb': btile(np.asarray(inputs['conv_pw1_b'], f32)[E:2 * E]),
        'dwcol': dwcol, 'bdw': btile(inputs['conv_dw_b']),
        'bdwm': btile(np.asarray(inputs['conv_dw_b'], f32) - 1.0),
        'w_pw2': t_(inputs['conv_pw2_w']), 'bp2': btile(inputs['conv_pw2_b']),
        'w_ff1': t_(inputs['ff_w1']), 'bg1': btile(inputs['ff_b1']),
        'bg1m': btile(np.asarray(inputs['ff_b1'], f32) - 1.0),
        'w_ff2': t_(inputs['ff_w2']), 'bg2': btile(inputs['ff_b2']),
        'eps_c': np.exp(np.asarray(inputs['norm_eps'], f32)).reshape(1, 1),
        'ones16': np.ones((1, 128), f16),
        'onescol16': np.ones((128, 1), f16),
        'ones32': np.ones((1, 128), f32),
        'ident16': np.eye(128, dtype=f16),
    }

    src_t = np.ascontiguousarray(src.transpose(1, 2, 0))  # (N, E, T)
    in_maps = []
    for c in range(NCORE):
        m = dict(common)
        m['xt'] = np.ascontiguousarray(
            src_t[NB * c:NB * (c + 1)].astype(f16))
        in_maps.append(m)
    return in_maps


def _run(inputs, trace=False):
    from concourse import bass_utils
    if 'nc1' not in _cached:
        _cached['nc1'] = _build()
    nc = _cached['nc1']
    in_maps = _prep_inputs(inputs)
    res = bass_utils.run_bass_kernel_spmd(nc, in_maps,
                                          core_ids=list(range(NCORE)),
                                          trace=trace)
    yts = np.stack([res.results[c]['yt'] for c in range(NCORE)])  # (8,2,E,T)
    out = np.ascontiguousarray(
        yts.transpose(3, 0, 1, 2).reshape(T, N, E)).astype(np.float32)
    return out, res


def kernel(**inputs):
    out, _ = _run(inputs, trace=False)
    return out


def _make_runner(inputs, repeat=1):
    """Build a zero-transfer on-device runner for timing.

    Mirrors bass2jax.run_bass_via_pjrt's shard_map setup but without buffer
    donation, so nothing is re-transferred between timed calls.
    """
    import jax
    import numpy as _np
    import concourse.mybir as mybir
    from concourse.bass2jax import (_bass_exec_p, install_neuronx_cc_hook,
                                    partition_id_tensor)
    from jax.experimental.shard_map import shard_map
    from jax.sharding import Mesh, PartitionSpec, NamedSharding

    key = f'nc{repeat}'
    if key not in _cached:
        _cached[key] = _build(repeat)
    nc = _cached[key]
    install_neuronx_cc_hook()
    in_maps = _prep_inputs(inputs)

    in_names, out_names, out_avals, zero_outs = [], [], [], []
    for alloc in nc.m.functions[0].allocations:
        if not isinstance(alloc, mybir.MemoryLocationSet):
            continue
        name = alloc.memorylocations[0].name
        if alloc.kind == "ExternalInput":
            if nc.partition_id_tensor is None or \
                    name != nc.partition_id_tensor.name:
                in_names.append(name)
        elif alloc.kind == "ExternalOutput":
            out_names.append(name)
            shape = tuple(alloc.tensor_shape)
            dtype = mybir.dt.np(alloc.dtype)
            out_avals.append(jax.core.ShapedArray(shape, dtype))
            zero_outs.append(_np.zeros(shape, dtype))
    n_params = len(in_names)
    all_names = in_names + out_names
    if nc.partition_id_tensor is not None:
        all_names = all_names + [nc.partition_id_tensor.name]

    def _body(*args):
        operands = list(args)
        if nc.partition_id_tensor is not None:
            operands.append(partition_id_tensor())
        outs = _bass_exec_p.bind(
            *operands, out_avals=tuple(out_avals), in_names=tuple(all_names),
            out_names=tuple(out_names), lowering_input_output_aliases=(),
            sim_require_finite=True, sim_require_nnan=True, nc=nc)
        return tuple(outs)

    devices = jax.devices()[:NCORE]
    mesh = Mesh(_np.asarray(devices), ("core",))
    spec = PartitionSpec("core")
    sharded = jax.jit(shard_map(
        _body, mesh=mesh, in_specs=(spec,) * (n_params + len(out_names)),
        out_specs=(spec,) * len(out_names), check_rep=False))
    sh = NamedSharding(mesh, spec)
    concat_in = [jax.device_put(
        _np.concatenate([_np.asarray(in_maps[c][nm]) for c in range(NCORE)],
                        axis=0), sh) for nm in in_names]
    concat_zero = [jax.device_put(
        _np.zeros((NCORE * z.shape[0], *z.shape[1:]), z.dtype), sh)
        for z in zero_outs]

    def run():
        out = sharded(*concat_in, *concat_zero)
        jax.block_until_ready(out)
        return out

    def gather(out):
        yts = _np.asarray(out[out_names.index('yt')]).reshape(
            NCORE, NB, E, T)
        return _np.ascontiguousarray(
            yts.transpose(3, 0, 1, 2).reshape(T, N, E)).astype(_np.float32)

    return run, gather


def _bench(inputs, iters=10, repeat=1):
    import time
    run, gather = _make_runner(inputs, repeat)
    out = run()
    times = []
    for _ in range(iters):
        t0 = time.perf_counter()
        out = run()
        times.append(time.perf_counter() - t0)
    return gather(out), times
